# revision 5
# baseline (speedup 1.0000x reference)
"""Distributed Bass/Tile kernel for EnhancedVariationalHierarchicalGAT on 8 trn2 cores.

Sharding: nodes split across 8 cores (6250 each). Edges partitioned by dst
shard, sorted by (dst-block, src-half). Per layer: each core builds its own
shard of the feature table (h = act @ W, plus per-node attention scores),
AllGathers the full table to every core, then gathers h[src] rows per edge
with dma_gather and aggregates per dst-block with select-matrix matmuls.
Per-dst softmax uses exp(lrelu(u+v)) == max(e^u e^v, e^{0.2u} e^{0.2v}).
BatchNorm stats are AllReduced. The VAE head runs on the own shard.
"""
import math
import numpy as np
import jax
import jax.numpy as jnp
from jax.sharding import Mesh, PartitionSpec
from jax.experimental.shard_map import shard_map

from concourse import bacc, mybir, tile, library_config
from concourse.bass2jax import (
    _bass_exec_p,
    install_neuronx_cc_hook,
    partition_id_tensor,
)

F32 = mybir.dt.float32
BF16 = mybir.dt.bfloat16
I16 = mybir.dt.int16
AX = mybir.AxisListType
OP = mybir.AluOpType
AF = mybir.ActivationFunctionType

NCORES = 8
H, C = 4, 64
HC = H * C          # 256
IN = 256
L, OUT, NCAUX = 32, 32, 30
ROWB = 384          # bf16 elems per table row (768 B): h(256) | al_s(4) | pad
BN_EPS = 1e-5


# ---------------------------------------------------------------- host prep
def prep_graph(edge_index, N):
    """Partition/sort/pad edges. Returns (meta, per_core list of dicts)."""
    S = N // NCORES
    NB = (S + 127) // 128
    HHALF = N // 2
    src = np.concatenate([edge_index[0], np.arange(N, dtype=np.int64)])
    dst = np.concatenate([edge_index[1], np.arange(N, dtype=np.int64)])
    src = src.astype(np.int64)
    dst = dst.astype(np.int64)

    per_core_raw = []
    counts = np.zeros((NCORES, NB, 2), np.int64)
    for k in range(NCORES):
        m = (dst // S) == k
        es, ed = src[m], dst[m] - k * S
        b = ed >> 7
        half = (es >= HHALF).astype(np.int64)
        order = np.lexsort((ed, half, b))
        es, ed, b, half = es[order], ed[order], b[order], half[order]
        cnt = np.zeros((NB, 2), np.int64)
        np.add.at(cnt, (b, half), 1)
        counts[k] = cnt
        per_core_raw.append((es, ed, b, half))

    cmax = counts.max(axis=0)                       # [NB, 2]
    T = np.maximum((cmax + 127) // 128, (cmax > 0).astype(np.int64))  # tiles
    # tile layout: for b in NB: for h in 2: T[b,h] tiles
    NT = int(T.sum())
    NI = NT * 128

    meta = dict(N=N, S=S, NB=NB, HHALF=HHALF, T=T, NT=NT, NI=NI)

    per_core = []
    for k in range(NCORES):
        es, ed, b, half = per_core_raw[k]
        idx_flat = np.zeros(NI, np.int16)
        dc = np.full((128, NT), -1.0, np.float32)
        # walk groups in (b, half) order; edges already sorted that way
        epos = 0
        tpos = 0
        for bb in range(NB):
            for hh in range(2):
                n = counts[k, bb, hh]
                ntile = int(T[bb, hh])
                if ntile == 0:
                    assert n == 0
                    continue
                sl = slice(epos, epos + n)
                loc = np.arange(n)
                gsrc = es[sl] - hh * HHALF
                gdl = ed[sl] - (bb << 7)
                base = tpos * 128
                idx_flat[base : base + n] = gsrc.astype(np.int16)
                # pad slots keep idx 0 (valid row, nullified by dc=-1)
                dc[loc % 128, tpos + loc // 128] = gdl
                epos += n
                tpos += ntile
        assert epos == len(es) and tpos == NT
        # wrap idx: slot i -> partition i%16, col i//16; replicate x8
        w = idx_flat.reshape(-1, 16).T               # [16, NI/16]
        idx_w = np.tile(w, (8, 1)).astype(np.int16)  # [128, NI/16]
        per_core.append(dict(idx=idx_w, dc=dc.astype(jnp.bfloat16)))
    return meta, per_core


def host_consts(inputs, meta):
    """Build replicated constant input arrays (same for every core)."""
    d = {}
    iota = np.tile(np.arange(128, dtype=np.float32)[None, :], (128, 1))
    d["iota_bf"] = iota.astype(jnp.bfloat16)
    d["iota_col"] = np.arange(128, dtype=np.float32)[:, None]  # unused spare
    d["ident_bf"] = np.eye(128, dtype=np.float32).astype(jnp.bfloat16)
    d["ident_f32"] = np.eye(128, dtype=np.float32)
    d["ones_col"] = np.ones((128, 1), np.float32)
    d["ones4"] = np.ones((4, 128), np.float32)
    for l, (ka, kd) in enumerate(
        [("asrc1", "adst1"), ("asrc2", "adst2"), ("asrc3", "adst3")], 1
    ):
        asrc = np.asarray(inputs[ka], np.float32).reshape(HC)   # [H,C] -> flat
        adst = np.asarray(inputs[kd], np.float32).reshape(HC)
        d[f"asrc_bc{l}"] = np.tile(asrc[None, :], (128, 1))
        d[f"adst_bc{l}"] = np.tile(adst[None, :], (128, 1))
        g = np.asarray(inputs[f"g{l}"], np.float32)
        be = np.asarray(inputs[f"be{l}"], np.float32)
        d[f"gcol{l}"] = np.stack([g[:128], g[128:]], axis=1)     # [128,2]
        d[f"bcol{l}"] = np.stack([be[:128], be[128:]], axis=1)
        d[f"W{l}"] = np.asarray(inputs[f"W{l}"], np.float32).reshape(2, 128, HC).transpose(1, 0, 2)
        d[f"bvec{l}"] = np.tile(np.asarray(inputs[f"b{l}"], np.float32)[None, :], (128, 1))
    d["Wr"] = np.asarray(inputs["Wr"], np.float32).reshape(2, 128, HC).transpose(1, 0, 2)
    d["Wmu"] = np.asarray(inputs["Wmu"], np.float32).reshape(2, 128, L).transpose(1, 0, 2)
    d["Wlv"] = np.asarray(inputs["Wlv"], np.float32).reshape(2, 128, L).transpose(1, 0, 2)
    d["Wa"] = np.asarray(inputs["Wa"], np.float32).reshape(2, 128, NCAUX).transpose(1, 0, 2)
    d["Wd"] = np.asarray(inputs["Wd"], np.float32)
    d["bmu_bc"] = np.tile(np.asarray(inputs["bmu"], np.float32)[None, :], (128, 1))
    d["blv_bc"] = np.tile(np.asarray(inputs["blv"], np.float32)[None, :], (128, 1))
    d["bd_bc"] = np.tile(np.asarray(inputs["bd"], np.float32)[None, :], (128, 1))
    d["ba_bc"] = np.tile(np.asarray(inputs["ba"], np.float32)[None, :], (128, 1))
    d["br_bc"] = np.tile(np.asarray(inputs["br"], np.float32)[None, :], (128, 1))
    return d


# ---------------------------------------------------------------- builder
def build_kernel(meta):
    N, S, NB, HHALF = meta["N"], meta["S"], meta["NB"], meta["HHALF"]
    T, NT, NI = meta["T"], meta["NT"], meta["NI"]
    TMAX = int(T.max()) if NT else 1

    nc = bacc.Bacc(
        "TRN2", target_bir_lowering=False, debug=False,
        num_devices=NCORES, num_swdge_queues=4,
    )
    P = lambda n, s, dt: nc.declare_dram_parameter(n, s, dt, isOutput=False)
    x_in = P("x", [S, IN], F32)
    eps_in = P("eps", [S, L], F32)
    idx_in = P("idx", [128, NI // 16], I16)
    dc_in = P("dc", [128, NT], BF16)
    iota_in = P("iota_bf", [128, 128], BF16)
    identbf_in = P("ident_bf", [128, 128], BF16)
    identf_in = P("ident_f32", [128, 128], F32)
    onescol_in = P("ones_col", [128, 1], F32)
    ones4_in = P("ones4", [4, 128], F32)
    wl_in = [P(f"W{l}", [128, 2, HC], F32) for l in (1, 2, 3)]
    asrc_in = [P(f"asrc_bc{l}", [128, HC], F32) for l in (1, 2, 3)]
    adst_in = [P(f"adst_bc{l}", [128, HC], F32) for l in (1, 2, 3)]
    gcol_in = [P(f"gcol{l}", [128, 2], F32) for l in (1, 2, 3)]
    bcol_in = [P(f"bcol{l}", [128, 2], F32) for l in (1, 2, 3)]
    bvec_in = [P(f"bvec{l}", [128, HC], F32) for l in (1, 2, 3)]
    wr_in = P("Wr", [128, 2, HC], F32)
    wmu_in = P("Wmu", [128, 2, L], F32)
    wlv_in = P("Wlv", [128, 2, L], F32)
    wa_in = P("Wa", [128, 2, NCAUX], F32)
    wd_in = P("Wd", [L, OUT], F32)
    bmu_in = P("bmu_bc", [128, L], F32)
    blv_in = P("blv_bc", [128, L], F32)
    bd_in = P("bd_bc", [128, OUT], F32)
    ba_in = P("ba_bc", [128, NCAUX], F32)
    br_in = P("br_bc", [128, HC], F32)

    O = lambda n, s: nc.declare_dram_parameter(n, s, F32, isOutput=True)
    out_o, mu_o, lv_o = O("out", [S, OUT]), O("mu", [S, L]), O("logvar", [S, L])
    aux_o, enc_o = O("aux", [S, NCAUX]), O("enc", [S, HC])

    NRl = S - (NB - 1) * 128  # rows in last block

    with tile.TileContext(nc) as tc:
        with (
            tc.tile_pool(name="dram", bufs=1, space="DRAM") as dram,
            tc.tile_pool(name="const", bufs=1) as cons,
            tc.tile_pool(name="big", bufs=1) as big,
            tc.tile_pool(name="gat", bufs=2) as gatp,
            tc.tile_pool(name="work", bufs=2) as work,
            tc.tile_pool(name="selp", bufs=3) as selp,
            tc.tile_pool(name="ps", bufs=2, space="PSUM") as ps,
            tc.tile_pool(name="ps1", bufs=1, space="PSUM") as ps1,
        ):
            tables = [
                dram.tile([N, ROWB], BF16, addr_space="Shared", name=f"table{i}")
                for i in range(3)
            ]
            own_rows_l = [
                dram.tile([S, ROWB], BF16, name=f"own_rows{i}") for i in range(3)
            ]
            stats_in_l = [
                dram.tile([128, 4], F32, name=f"stats_in{i}") for i in range(3)
            ]
            stats_out_l = [
                dram.tile([128, 4], F32, name=f"stats_out{i}") for i in range(3)
            ]

            # ---- load constants
            def ld(pool, src, shape, dt, name):
                t = pool.tile(shape, dt, name=name)
                nc.sync.dma_start(t[:], src[:])
                return t

            idx_sb = ld(cons, idx_in, [128, NI // 16], I16, "idx_sb")
            dc_sb = ld(cons, dc_in, [128, NT], BF16, "dc_sb")
            iota_sb = ld(cons, iota_in, [128, 128], BF16, "iota_sb")
            identbf = ld(cons, identbf_in, [128, 128], BF16, "identbf")
            identf = ld(cons, identf_in, [128, 128], F32, "identf")
            onescol = ld(cons, onescol_in, [128, 1], F32, "onescol")
            ones4 = ld(cons, ones4_in, [4, 128], F32, "ones4")
            W_sb = [ld(cons, wl_in[i], [128, 2, HC], F32, f"W{i}sb") for i in range(3)]
            asrc_sb = [ld(cons, asrc_in[i], [128, HC], F32, f"as{i}sb") for i in range(3)]
            adst_sb = [ld(cons, adst_in[i], [128, HC], F32, f"ad{i}sb") for i in range(3)]
            gcol_sb = [ld(cons, gcol_in[i], [128, 2], F32, f"gc{i}sb") for i in range(3)]
            bcol_sb = [ld(cons, bcol_in[i], [128, 2], F32, f"bc{i}sb") for i in range(3)]
            bvec_sb = [ld(cons, bvec_in[i], [128, HC], F32, f"bv{i}sb") for i in range(3)]
            Wr_sb = ld(cons, wr_in, [128, 2, HC], F32, "Wrsb")
            Wmu_sb = ld(cons, wmu_in, [128, 2, L], F32, "Wmusb")
            Wlv_sb = ld(cons, wlv_in, [128, 2, L], F32, "Wlvsb")
            Wa_sb = ld(cons, wa_in, [128, 2, NCAUX], F32, "Wasb")
            Wd_sb = ld(cons, wd_in, [L, OUT], F32, "Wdsb")
            bmu_sb = ld(cons, bmu_in, [128, L], F32, "bmusb")
            blv_sb = ld(cons, blv_in, [128, L], F32, "blvsb")
            bd_sb = ld(cons, bd_in, [128, OUT], F32, "bdsb")
            ba_sb = ld(cons, ba_in, [128, NCAUX], F32, "basb")
            br_sb = ld(cons, br_in, [128, HC], F32, "brsb")

            nc.gpsimd.load_library(library_config.mlp)

            gat_sb = big.tile([128, NB, HC], F32, name="gat_sb")
            evv_sb = big.tile([128, NB, 8], F32, name="evv_sb")
            stats_acc = big.tile([128, 4], F32, name="stats_acc")
            sc_bc = big.tile([128, HC], F32, name="sc_bc")
            bi_bc = big.tile([128, HC], F32, name="bi_bc")

            qctr = [0]

            def nr_of(b):
                return 128 if b < NB - 1 else NRl

            # ------------------------------------------------ table row build
            def build_rows(b, src_ap, li):
                """src_ap: [NR, 256] f32 SBUF acts; computes table row block b
                for layer li (0-based): h = src @ W[li], al_s/al_d, evv."""
                NR = nr_of(b)
                xT = work.tile([128, 2, 128], F32, name=f"xT_{li}_{b}", tag="xT")
                for k in range(2):
                    tp = ps.tile([128, 128], F32, name=f"tp{li}_{b}_{k}", tag="t1")
                    nc.tensor.transpose(tp[:, 0:NR], src_ap[0:NR, k * 128:(k + 1) * 128], identf[0:NR, 0:NR])
                    nc.scalar.copy(xT[:, k, 0:NR], tp[:, 0:NR])
                hps = ps.tile([128, HC], F32, name=f"h{li}_{b}", tag="t2")
                for k in range(2):
                    nc.tensor.matmul(
                        hps[0:NR, :], xT[:, k, 0:NR], W_sb[li][:, k, :],
                        start=(k == 0), stop=(k == 1), skip_group_check=True,
                    )
                tmp = work.tile([128, HC], F32, name=f"tmp{li}_{b}", tag="tmp")
                als = work.tile([128, 4], F32, name=f"als{li}_{b}", tag="als")
                ald = work.tile([128, 4], F32, name=f"ald{li}_{b}", tag="ald")
                nc.vector.tensor_tensor(tmp[0:NR, :], hps[0:NR, :], asrc_sb[li][0:NR, :], OP.mult)
                nc.vector.tensor_reduce(als[0:NR, :], tmp[0:NR, :].rearrange("p (h c) -> p h c", h=H), AX.X, OP.add)
                nc.vector.tensor_tensor(tmp[0:NR, :], hps[0:NR, :], adst_sb[li][0:NR, :], OP.mult)
                nc.vector.tensor_reduce(ald[0:NR, :], tmp[0:NR, :].rearrange("p (h c) -> p h c", h=H), AX.X, OP.add)
                nc.scalar.activation(evv_sb[0:NR, b, 0:4], ald[0:NR, :], AF.Exp)
                nc.scalar.activation(evv_sb[0:NR, b, 4:8], ald[0:NR, :], AF.Exp, scale=0.2)
                th = work.tile([128, ROWB], BF16, name=f"th{li}_{b}", tag="th")
                nc.scalar.copy(th[0:NR, 0:HC], hps[0:NR, :])
                nc.vector.tensor_copy(th[0:NR, HC:HC + 8].bitcast(F32), als[0:NR, :])
                nc.vector.memset(th[0:NR, HC + 8:ROWB], 0)
                nc.sync.dma_start(own_rows_l[li][b * 128: b * 128 + NR, :], th[0:NR, :])

            # ------------------------------------------------ one GAT layer
            def edge_phase(li):
                nc.gpsimd.collective_compute(
                    "AllGather", OP.bypass,
                    replica_groups=[list(range(NCORES))],
                    ins=[own_rows_l[li].opt()], outs=[tables[li].opt()],
                )
                tpos = 0
                for b in range(NB):
                    pblk = ps.tile([128, HC + 4], F32, name=f"pb{li}_{b}", tag="t3")
                    first = True
                    ntot = int(T[b, 0] + T[b, 1])
                    done = 0
                    for hh in range(2):
                        Tb = int(T[b, hh])
                        if Tb == 0:
                            continue
                        g = gatp.tile([128, TMAX, ROWB], BF16, name=f"g{li}_{b}_{hh}", tag="g")
                        nc.gpsimd.dma_gather(
                            out_ap=g[:, 0:Tb, :],
                            in_ap=tables[li][hh * HHALF: hh * HHALF + (N - HHALF if hh else HHALF), :],
                            idxs_ap=idx_sb[:, tpos * 8: (tpos + Tb) * 8],
                            num_idxs=Tb * 128, num_idxs_reg=Tb * 128,
                            elem_size=ROWB, single_packet=False,
                            queue_num=qctr[0] % 4,
                        )
                        qctr[0] += 1
                        expu = work.tile([128, TMAX, 8], F32, name=f"eu{li}_{b}_{hh}", tag="expu")
                        gals = g[:, 0:Tb, HC:HC + 8].bitcast(F32)
                        nc.scalar.activation(expu[:, 0:Tb, 0:4], gals, AF.Exp)
                        nc.scalar.activation(expu[:, 0:Tb, 4:8], gals, AF.Exp, scale=0.2)
                        selr = work.tile([128, TMAX, 128], BF16, name=f"sr{li}_{b}_{hh}", tag="selr")
                        dcs = dc_sb[:, tpos: tpos + Tb]
                        nc.vector.tensor_tensor(
                            selr[:, 0:Tb, :],
                            iota_sb.unsqueeze(1).broadcast_to([128, Tb, 128]),
                            dcs.unsqueeze(2).broadcast_to([128, Tb, 128]),
                            OP.is_equal,
                        )
                        pev = ps.tile([128, TMAX, 8], F32, name=f"pe{li}_{b}_{hh}", tag="t4")
                        for t in range(Tb):
                            stp = ps.tile([128, 128], BF16, name=f"st{li}_{b}_{hh}_{t}", tag="t1")
                            nc.tensor.transpose(stp[:], selr[:, t, :], identbf[:])
                            stsb = selp.tile([128, 128], F32, name=f"ss{li}_{b}_{hh}_{t}", tag="ss")
                            nc.scalar.copy(stsb[:], stp[:])
                            nc.tensor.matmul(
                                pev[:, t, :], stsb[:], evv_sb[:, b, :],
                                start=True, stop=True, skip_group_check=True,
                            )
                        mrun = work.tile([128, TMAX, 8], F32, name=f"mr{li}_{b}_{hh}", tag="mr")
                        nc.vector.tensor_tensor(mrun[:, 0:Tb, :], expu[:, 0:Tb, :], pev[:, 0:Tb, :], OP.mult)
                        expw = work.tile([128, TMAX, 4], F32, name=f"ew{li}_{b}_{hh}", tag="ew")
                        nc.vector.tensor_tensor(expw[:, 0:Tb, :], mrun[:, 0:Tb, 0:4], mrun[:, 0:Tb, 4:8], OP.max)
                        rhs = work.tile([128, TMAX, HC + 8], BF16, name=f"rh{li}_{b}_{hh}", tag="rhs")
                        nc.vector.tensor_tensor(
                            rhs[:, 0:Tb, 0:HC].rearrange("p t (h c) -> p t h c", h=H),
                            g[:, 0:Tb, 0:HC].rearrange("p t (h c) -> p t h c", h=H),
                            expw[:, 0:Tb, :].unsqueeze(3).broadcast_to([128, Tb, H, C]),
                            OP.mult,
                        )
                        nc.vector.tensor_copy(rhs[:, 0:Tb, HC:HC + 4], expw[:, 0:Tb, :])
                        for t in range(Tb):
                            done += 1
                            nc.tensor.matmul(
                                pblk[:], selr[:, t, :], rhs[:, t, 0:HC + 4],
                                start=first, stop=(done == ntot),
                                skip_group_check=True,
                            )
                            first = False
                        tpos += Tb
                    # finalize block
                    NR = nr_of(b)
                    rec = work.tile([128, 4], F32, name=f"rc{li}_{b}", tag="rec")
                    nc.vector.reciprocal(rec[:], pblk[:, HC:HC + 4])
                    nc.vector.tensor_tensor(
                        gat_sb[:, b, :].rearrange("p (h c) -> p h c", h=H),
                        pblk[:, 0:HC].rearrange("p (h c) -> p h c", h=H),
                        rec.unsqueeze(2).broadcast_to([128, H, C]),
                        OP.mult,
                    )
                    # bias (zero-sum through BN, but apply for generality)
                    nc.vector.tensor_tensor(gat_sb[:, b, :], gat_sb[:, b, :], bvec_sb[li], OP.add)
                    # BN partial sums
                    sq = work.tile([128, HC], F32, name=f"sq{li}_{b}", tag="sq")
                    nc.scalar.activation(sq[0:NR, :], gat_sb[0:NR, b, :], AF.Square)
                    stp4 = ps.tile([128, 4], F32, name=f"sp{li}_{b}", tag="t4")
                    for j, (srct, kk) in enumerate(
                        [(gat_sb[:, b, :], 0), (gat_sb[:, b, :], 1), (sq, 0), (sq, 1)]
                    ):
                        nc.tensor.matmul(
                            stp4[:, j:j + 1], srct[0:NR, kk * 128:(kk + 1) * 128],
                            onescol[0:NR, :], start=True, stop=True,
                            skip_group_check=True,
                        )
                    if b == 0:
                        nc.vector.tensor_copy(stats_acc[:], stp4[:])
                    else:
                        nc.vector.tensor_tensor(stats_acc[:], stats_acc[:], stp4[:], OP.add)
                assert tpos == NT

            def bn_normalize(li):
                """AllReduce stats -> scale/bias bcast tiles -> norm+relu in gat_sb."""
                nc.sync.dma_start(stats_in_l[li][:], stats_acc[:])
                nc.gpsimd.collective_compute(
                    "AllReduce", OP.add,
                    replica_groups=[list(range(NCORES))],
                    ins=[stats_in_l[li].opt()], outs=[stats_out_l[li].opt()],
                )
                stg = work.tile([128, 4], F32, name=f"stg{li}", tag="stg")
                nc.sync.dma_start(stg[:], stats_out_l[li][:])
                inv = 1.0 / float(N)
                p4 = work.tile([128, 4], F32, name=f"p4_{li}", tag="p4")
                mcol = work.tile([128, 2], F32, name=f"mc{li}", tag="mc")
                var = work.tile([128, 2], F32, name=f"vr{li}", tag="vr")
                nc.scalar.mul(mcol[:], stg[:, 0:2], inv)
                nc.scalar.mul(var[:], stg[:, 2:4], inv)          # E[x^2]
                t2 = work.tile([128, 2], F32, name=f"t2_{li}", tag="t2w")
                nc.vector.tensor_tensor(t2[:], mcol[:], mcol[:], OP.mult)
                nc.vector.tensor_tensor(var[:], var[:], t2[:], OP.subtract)
                nc.vector.tensor_scalar_add(var[:], var[:], BN_EPS)
                nc.scalar.activation(var[:], var[:], AF.Sqrt)
                nc.vector.reciprocal(var[:], var[:])             # 1/sqrt(var+eps)
                nc.vector.tensor_tensor(p4[:, 0:2], gcol_sb[li][:], var[:], OP.mult)
                nc.vector.tensor_tensor(t2[:], mcol[:], p4[:, 0:2], OP.mult)
                nc.vector.tensor_tensor(p4[:, 2:4], bcol_sb[li][:], t2[:], OP.subtract)
                # broadcast to [128, 256] row tiles via transpose + outer
                for j, dstap in enumerate(
                    [sc_bc[:, 0:128], sc_bc[:, 128:256], bi_bc[:, 0:128], bi_bc[:, 128:256]]
                ):
                    tpj = ps.tile([1, 128], F32, name=f"tj{li}_{j}", tag="t1")
                    nc.tensor.transpose(tpj[:], p4[:, j:j + 1], identf[:])
                    rowj = work.tile([1, 128], F32, name=f"rj{li}_{j}", tag="rowj")
                    nc.scalar.copy(rowj[:], tpj[:])
                    obc = ps.tile([128, 128], F32, name=f"ob{li}_{j}", tag="t2")
                    nc.tensor.matmul(obc[:], ones4[0:1, :], rowj[:], start=True, stop=True, skip_group_check=True)
                    nc.scalar.copy(dstap, obc[:])
                # normalize + relu (in place in gat_sb)
                for b in range(NB):
                    t1 = work.tile([128, HC], F32, name=f"n1{li}_{b}", tag="n1")
                    nc.vector.tensor_tensor(t1[:], gat_sb[:, b, :], sc_bc[:], OP.mult)
                    nc.vector.tensor_tensor(t1[:], t1[:], bi_bc[:], OP.add)
                    nc.scalar.activation(gat_sb[:, b, :], t1[:], AF.Relu)

            # ================================================== layer 1 table
            for b in range(NB):
                NR = nr_of(b)
                xblk = work.tile([128, IN], F32, name=f"x0_{b}", tag="xblk")
                nc.sync.dma_start(xblk[0:NR, :], x_in[b * 128: b * 128 + NR, :])
                build_rows(b, xblk, 0)

            for li in range(3):
                edge_phase(li)
                bn_normalize(li)
                if li < 2:
                    for b in range(NB):
                        build_rows(b, gat_sb[:, b, :], li + 1)

            # ================================================== VAE head
            for b in range(NB):
                NR = nr_of(b)
                xblk = work.tile([128, IN], F32, name=f"xh_{b}", tag="xblk")
                nc.sync.dma_start(xblk[0:NR, :], x_in[b * 128: b * 128 + NR, :])
                xT = work.tile([128, 2, 128], F32, name=f"xTh_{b}", tag="xT")
                for k in range(2):
                    tp = ps.tile([128, 128], F32, name=f"tph_{b}_{k}", tag="t1")
                    nc.tensor.transpose(tp[:, 0:NR], xblk[0:NR, k * 128:(k + 1) * 128], identf[0:NR, 0:NR])
                    nc.scalar.copy(xT[:, k, 0:NR], tp[:, 0:NR])
                res = ps.tile([128, HC], F32, name=f"res_{b}", tag="t3")
                for k in range(2):
                    nc.tensor.matmul(res[0:NR, :], xT[:, k, 0:NR], Wr_sb[:, k, :],
                                     start=(k == 0), stop=(k == 1), skip_group_check=True)
                enc = work.tile([128, HC], F32, name=f"enc_{b}", tag="enc")
                nc.vector.tensor_tensor(enc[0:NR, :], gat_sb[0:NR, b, :], res[0:NR, :], OP.add)
                nc.vector.tensor_tensor(enc[0:NR, :], enc[0:NR, :], br_sb[0:NR, :], OP.add)
                nc.sync.dma_start(enc_o[b * 128: b * 128 + NR, :], enc[0:NR, :])
                eT = work.tile([128, 2, 128], F32, name=f"eT_{b}", tag="eT")
                for k in range(2):
                    tp = ps.tile([128, 128], F32, name=f"tpe_{b}_{k}", tag="t1")
                    nc.tensor.transpose(tp[:, 0:NR], enc[0:NR, k * 128:(k + 1) * 128], identf[0:NR, 0:NR])
                    nc.scalar.copy(eT[:, k, 0:NR], tp[:, 0:NR])

                def head_mm(Wt, ncols, tag):
                    p = ps.tile([128, ncols], F32, name=f"hm{tag}_{b}", tag="t4")
                    for k in range(2):
                        nc.tensor.matmul(p[0:NR, :], eT[:, k, 0:NR], Wt[:, k, :],
                                         start=(k == 0), stop=(k == 1), skip_group_check=True)
                    return p

                mups = head_mm(Wmu_sb, L, "mu")
                mu = work.tile([128, L], F32, name=f"mu_{b}", tag="mu")
                nc.vector.tensor_tensor(mu[0:NR, :], mups[0:NR, :], bmu_sb[0:NR, :], OP.add)
                nc.sync.dma_start(mu_o[b * 128: b * 128 + NR, :], mu[0:NR, :])
                lvps = head_mm(Wlv_sb, L, "lv")
                lv = work.tile([128, L], F32, name=f"lv_{b}", tag="lv")
                nc.vector.tensor_tensor(lv[0:NR, :], lvps[0:NR, :], blv_sb[0:NR, :], OP.add)
                nc.sync.dma_start(lv_o[b * 128: b * 128 + NR, :], lv[0:NR, :])
                sd = work.tile([128, L], F32, name=f"sd_{b}", tag="sd")
                nc.scalar.activation(sd[0:NR, :], lv[0:NR, :], AF.Exp, scale=0.5)
                epsb = work.tile([128, L], F32, name=f"ep_{b}", tag="ep")
                nc.sync.dma_start(epsb[0:NR, :], eps_in[b * 128: b * 128 + NR, :])
                z = work.tile([128, L], F32, name=f"z_{b}", tag="z")
                nc.vector.tensor_tensor(z[0:NR, :], epsb[0:NR, :], sd[0:NR, :], OP.mult)
                nc.vector.tensor_tensor(z[0:NR, :], z[0:NR, :], mu[0:NR, :], OP.add)
                tpz = ps.tile([L, 128], F32, name=f"tpz_{b}", tag="t1")
                nc.tensor.transpose(tpz[:, 0:NR], z[0:NR, :], identf[0:NR, 0:NR])
                zT = work.tile([L, 128], F32, name=f"zT_{b}", tag="zT")
                nc.scalar.copy(zT[:, 0:NR], tpz[:, 0:NR])
                ops = ps.tile([128, OUT], F32, name=f"op_{b}", tag="t4")
                nc.tensor.matmul(ops[0:NR, :], zT[:, 0:NR], Wd_sb[:], start=True, stop=True, skip_group_check=True)
                ob = work.tile([128, OUT], F32, name=f"ob_{b}", tag="ob")
                nc.vector.tensor_tensor(ob[0:NR, :], ops[0:NR, :], bd_sb[0:NR, :], OP.add)
                nc.sync.dma_start(out_o[b * 128: b * 128 + NR, :], ob[0:NR, :])
                axps = head_mm(Wa_sb, NCAUX, "ax")
                ax = work.tile([128, NCAUX], F32, name=f"ax_{b}", tag="ax")
                nc.vector.tensor_tensor(ax[0:NR, :], axps[0:NR, :], ba_sb[0:NR, :], OP.add)
                nc.sync.dma_start(aux_o[b * 128: b * 128 + NR, :], ax[0:NR, :])

    nc.finalize()
    return nc


# ---------------------------------------------------------------- runner
class BassRunner:
    def __init__(self, nc, n_cores):
        install_neuronx_cc_hook()
        self.nc, self.n_cores = nc, n_cores
        pname = nc.partition_id_tensor.name if nc.partition_id_tensor else None
        in_names, out_names, out_avals, zero_outs = [], [], [], []
        for alloc in nc.m.functions[0].allocations:
            if not isinstance(alloc, mybir.MemoryLocationSet):
                continue
            name = alloc.memorylocations[0].name
            if alloc.kind == "ExternalInput":
                if name != pname:
                    in_names.append(name)
            elif alloc.kind == "ExternalOutput":
                shape = tuple(alloc.tensor_shape)
                dtype = mybir.dt.np(alloc.dtype)
                out_names.append(name)
                out_avals.append(jax.core.ShapedArray(shape, dtype))
                zero_outs.append(np.zeros(shape, dtype))
        self.in_names, self.out_names = in_names, out_names
        self.out_avals, self.zero_outs = out_avals, zero_outs
        n_params, n_outs = len(in_names), len(out_names)
        all_in = in_names + out_names + ([pname] if pname else [])

        def _body(*args):
            operands = list(args)
            if pname is not None:
                operands.append(partition_id_tensor())
            return tuple(_bass_exec_p.bind(
                *operands, out_avals=tuple(out_avals), in_names=tuple(all_in),
                out_names=tuple(out_names), lowering_input_output_aliases=(),
                sim_require_finite=True, sim_require_nnan=True, nc=nc,
            ))

        devices = jax.devices()[:n_cores]
        self.mesh = Mesh(np.asarray(devices), ("core",))
        self._fn = jax.jit(
            shard_map(_body, mesh=self.mesh,
                      in_specs=(PartitionSpec("core"),) * (n_params + n_outs),
                      out_specs=(PartitionSpec("core"),) * n_outs,
                      check_rep=False),
            keep_unused=True,
        )

    def stage(self, in_maps):
        concat_in = [
            np.concatenate([np.asarray(m[n]) for m in in_maps], axis=0)
            for n in self.in_names
        ]
        concat_zeros = [
            np.zeros((self.n_cores * z.shape[0], *z.shape[1:]), z.dtype)
            for z in self.zero_outs
        ]
        sh = jax.sharding.NamedSharding(self.mesh, PartitionSpec("core"))
        self._dev = [jax.device_put(a, sh) for a in concat_in + concat_zeros]

    def run(self):
        outs = self._fn(*self._dev)
        jax.block_until_ready(outs)
        return outs

    def results(self, outs):
        res = []
        for c in range(self.n_cores):
            d = {}
            for i, n in enumerate(self.out_names):
                a = np.asarray(outs[i]).reshape(self.n_cores, *self.out_avals[i].shape)
                d[n] = a[c]
            res.append(d)
        return res


# ---------------------------------------------------------------- entry
_CACHE = {}


def kernel(**inputs):
    x = np.asarray(inputs["x"], np.float32)
    ei = np.asarray(inputs["edge_index"])
    eps = np.asarray(inputs["eps"], np.float32)
    N = x.shape[0]
    S = N // NCORES

    key = (N, ei.shape[1])
    if key not in _CACHE:
        meta, per_core = prep_graph(ei, N)
        nc = build_kernel(meta)
        runner = BassRunner(nc, NCORES)
        _CACHE[key] = (meta, per_core, runner)
    meta, per_core, runner = _CACHE[key]

    consts = host_consts(inputs, meta)
    in_maps = []
    for k in range(NCORES):
        m = dict(consts)
        m["x"] = x[k * S:(k + 1) * S]
        m["eps"] = eps[k * S:(k + 1) * S]
        m["idx"] = per_core[k]["idx"]
        m["dc"] = per_core[k]["dc"]
        in_maps.append(m)
    runner.stage(in_maps)
    outs = runner.run()
    res = runner.results(outs)
    full = {
        n: np.concatenate([r[n] for r in res], axis=0)
        for n in runner.out_names
    }
    return (full["out"], full["mu"], full["logvar"], full["aux"], full["enc"])
